# revision 17
# baseline (speedup 1.0000x reference)
"""Trainium2 Bass kernel for nn_Network_80049600463282.

LSTM language model: B=16, T=512, V=4096 (one-hot input), H=512 LSTM,
then MLP 512 -> 200 (relu) -> 4096, with fixed batch-norm scaling.

Strategy (8 NeuronCores, SPMD, zero collectives):
  - Data-parallel over batch: 2 examples per core.
  - One-hot @ W_x == embedding row gather -> precomputed host-side into the
    exact transposed SBUF layout the recurrence consumes (lstm bias + forget
    bias folded in).
  - Recurrence in transposed layout (gate dims on partitions): per step,
    64 bf16 matmuls [128x128 stationary W_h tile] x [128x2 moving h^T],
    PSUM-accumulated per 128-dim output chunk; sigmoid/tanh on ScalarE,
    cell update on VectorE, chunked x4 so gate latency hides under the
    next chunk's weight streaming.
  - hs^T accumulates in SBUF in matmul-ready layout; MLP runs at the end
    with BN scales folded into W1/W2 and b2 folded in via K=201 augmentation.
"""

import os
import numpy as np
import ml_dtypes

V = 4096
B = 16
T = int(os.environ.get("LSTM_KERNEL_T", "512"))
H = 512
DH = 200
N_CORES = 8
BL = 2  # examples per core
BN_S = 1.0 / np.sqrt(1.0 + 0.001)
# gate order inside a chunk: (i, f, o, j); column base offsets in the fused
# [*, 4H] kernel layout where reference order is i, j, f, o
GATE_BASE = [0, 2 * H, 3 * H, 1 * H]
SLOT = T  # h history slots per chunk (h_t at slot t; t=0 is special-cased)

_CACHE = {}


def _build_program():
    from concourse import bacc
    import concourse.mybir as mybir
    from concourse.tile import TileContext

    f32 = mybir.dt.float32
    bf16 = mybir.dt.bfloat16
    AFT = mybir.ActivationFunctionType

    nc = bacc.Bacc(target_bir_lowering=False)

    e_d = nc.declare_dram_parameter("e", [128, T * 32], bf16, False)
    wh_d = nc.declare_dram_parameter("wh", [128, 64 * 128], bf16, False)
    w1_d = nc.declare_dram_parameter("w1", [128, 4 * DH], bf16, False)
    w2_d = nc.declare_dram_parameter("w2", [128, 2 * 4096], bf16, False)
    b1_d = nc.declare_dram_parameter("b1v", [128, 2], f32, False)
    b2_d = nc.declare_dram_parameter("b2v", [128, 32], f32, False)
    out_d = nc.declare_dram_parameter("out", [4096, 2 * T], f32, isOutput=True)

    with TileContext(nc) as tc:
        with tc.sbuf_pool(name="const", bufs=1) as cpool:
            e_sb = cpool.tile([128, T * 32], bf16, name="e_sb")
            wh_sb = cpool.tile([128, 64 * 128], bf16, name="wh_sb")
            w1_sb = cpool.tile([128, 4 * DH], bf16, name="w1_sb")
            w2_sb = cpool.tile([128, 2 * 4096], bf16, name="w2_sb")
            b1_sb = cpool.tile([128, 2], f32, name="b1_sb")
            b2_sb = cpool.tile([128, 32], f32, name="b2_sb")
            # persistent state
            hst = cpool.tile([128, 4 * SLOT * 2], bf16, name="hst")
            cst = cpool.tile([128, 8], f32, name="cst")
            h1t = cpool.tile([128, 2048], bf16, name="h1t")

            nc.sync.dma_start(out=wh_sb[:, :], in_=wh_d[:, :])
            nc.sync.dma_start(out=e_sb[:, :], in_=e_d[:, :])
            nc.sync.dma_start(out=w1_sb[:, :], in_=w1_d[:, :])
            nc.sync.dma_start(out=w2_sb[:, :], in_=w2_d[:, :])
            nc.sync.dma_start(out=b1_sb[:, :], in_=b1_d[:, :])
            nc.sync.dma_start(out=b2_sb[:, :], in_=b2_d[:, :])

            # Engine-clock fences: each absorbs one input-DMA dependency into
            # an engine's observed clock so per-iteration ops carry at most a
            # single semaphore wait (walrus embedded-sync budget).
            fence = cpool.tile([128, 4], f32, name="fence")
            nc.vector.tensor_copy(fence[:, 0:1], e_sb[:, 0:1])
            nc.vector.tensor_copy(fence[:, 1:2], e_sb[:, T * 32 - 1: T * 32])
            nc.vector.tensor_copy(fence[:, 2:3], b2_sb[:, 0:1])
            nc.scalar.add(fence[:, 3:4], b1_sb[:, 0:1], 0.0)
            nc.tensor.ldweights(wh_sb[:, 0:128])

            with tc.psum_pool(name="zp", bufs=2) as zpool, \
                    tc.sbuf_pool(name="gw", bufs=3) as gpool:
                for t in range(T):
                    for c in range(4):
                        sif = gpool.tile([128, 6], f32, tag=f"s{c}", name=f"sif{c}")
                        tj = gpool.tile([128, 2], f32, tag=f"tj{c}", name=f"tj{c}")
                        if t == 0:
                            # z_0 = E_0 (h_{-1} = 0): no matmuls, gates read E
                            nc.scalar.activation(
                                sif[:, :], e_sb[:, c * 8: c * 8 + 6], AFT.Sigmoid
                            )
                            nc.scalar.activation(
                                tj[:, :], e_sb[:, c * 8 + 6: c * 8 + 8], AFT.Tanh
                            )
                            t1 = gpool.tile([128, 2], f32, tag=f"t1{c}", name=f"t1{c}")
                            nc.vector.tensor_mul(t1[:, :], sif[:, 0:2], tj[:, :])
                            # c_0 = sig(i)*tanh(j)  (cell state starts at 0)
                            nc.vector.tensor_copy(cst[:, c * 2:c * 2 + 2], t1[:, :])
                        else:
                            zp = zpool.tile([128, 8], f32, tag=f"z{c}", name=f"zp{c}")
                            # one accumulation group per PSUM bank: 16 matmuls,
                            # start only on the very first (bank-wide
                            # has_written clear); per-element semantics handle
                            # the interleaved column ranges.
                            for ck in range(4):
                                rhs = hst[:, ck * SLOT * 2 + (t - 1) * 2: ck * SLOT * 2 + (t - 1) * 2 + 2]
                                for g in range(4):
                                    slot = (c * 16 + ck * 4 + g) * 128
                                    nc.tensor.matmul(
                                        zp[:, g * 2:(g + 1) * 2],
                                        wh_sb[:, slot:slot + 128],
                                        rhs,
                                        start=(ck == 0 and g == 0),
                                        stop=(ck == 3 and g == 3),
                                    )
                            zf = gpool.tile([128, 8], f32, tag=f"zf{c}", name=f"zf{c}")
                            nc.vector.tensor_add(
                                zf[:, :], zp[:, :], e_sb[:, t * 32 + c * 8: t * 32 + c * 8 + 8]
                            )
                            nc.scalar.activation(sif[:, :], zf[:, 0:6], AFT.Sigmoid)
                            t2 = gpool.tile([128, 2], f32, tag=f"t2{c}", name=f"t2{c}")
                            nc.vector.tensor_mul(t2[:, :], cst[:, c * 2:c * 2 + 2], sif[:, 2:4])
                            nc.scalar.activation(tj[:, :], zf[:, 6:8], AFT.Tanh)
                            t1 = gpool.tile([128, 2], f32, tag=f"t1{c}", name=f"t1{c}")
                            nc.vector.tensor_mul(t1[:, :], sif[:, 0:2], tj[:, :])
                            nc.vector.tensor_add(cst[:, c * 2:c * 2 + 2], t1[:, :], t2[:, :])
                        tcs = gpool.tile([128, 2], f32, tag=f"tc{c}", name=f"tcs{c}")
                        nc.scalar.activation(tcs[:, :], cst[:, c * 2:c * 2 + 2], AFT.Tanh)
                        nc.vector.tensor_mul(
                            hst[:, c * SLOT * 2 + t * 2: c * SLOT * 2 + t * 2 + 2],
                            tcs[:, :],
                            sif[:, 4:6],
                        )

            # ---- MLP ----
            NT = 2 * T  # rows per core
            NCH = NT // 512 if NT >= 512 else 1
            NF = min(512, NT)
            # PE absorber for the w1 DMA dep
            nc.tensor.ldweights(w1_sb[:, 0:128])
            with tc.psum_pool(name="m1p", bufs=1) as m1pool:
                for m in range(2):
                    pm = 128 if m == 0 else DH - 128
                    for n in range(NCH):
                        ps = m1pool.tile([128, NF], f32, tag=f"h1{m}{n}", name=f"h1ps{m}{n}")
                        for c in range(4):
                            nc.tensor.matmul(
                                ps[0:pm, :],
                                w1_sb[:, c * DH + m * 128: c * DH + m * 128 + pm],
                                hst[:, c * SLOT * 2 + n * NF: c * SLOT * 2 + n * NF + NF],
                                start=(c == 0),
                                stop=(c == 3),
                            )
                        nc.scalar.activation(
                            h1t[0:pm, m * 1024 + n * NF: m * 1024 + n * NF + NF],
                            ps[0:pm, :],
                            AFT.Relu,
                            bias=b1_sb[0:pm, m:m + 1],
                        )

            # PE absorber for the w2 DMA dep
            nc.tensor.ldweights(w2_sb[:, 0:128])
            with tc.psum_pool(name="m2p", bufs=4) as m2pool, \
                    tc.sbuf_pool(name="ob", bufs=4) as opool:
                for mi in range(32):
                    for n in range(NCH):
                        ps2 = m2pool.tile([128, NF], f32, tag="o2", name=f"o2ps{mi}{n}")
                        nc.tensor.matmul(
                            ps2[:, :],
                            w2_sb[:, mi * 128: mi * 128 + 128],
                            h1t[0:128, n * NF: n * NF + NF],
                            start=True, stop=False,
                        )
                        nc.tensor.matmul(
                            ps2[:, :],
                            w2_sb[0:72, 4096 + mi * 128: 4096 + mi * 128 + 128],
                            h1t[0:72, 1024 + n * NF: 1024 + n * NF + NF],
                            start=False, stop=True,
                        )
                        ob = opool.tile([128, NF], f32, tag="ob", name=f"ob{mi}{n}")
                        # 1-elem memset absorbs the WAR on the outbound-DMA
                        # queue before the real copy (keeps copy at 1 wait)
                        nc.vector.memset(ob[0:1, 0:1], 0.0)
                        nc.vector.tensor_scalar_add(ob[:, :], ps2[:, :], b2_sb[:, mi:mi + 1])
                        nc.sync.dma_start(
                            out=out_d[mi * 128:(mi + 1) * 128, n * NF: n * NF + NF],
                            in_=ob[:, :],
                        )
    nc.finalize()
    return nc


def _prep_host(tokens, lstm_kernel, lstm_bias, W1, b1, W2, b2):
    """Build per-core input arrays in the packed layouts the program expects."""
    bf = ml_dtypes.bfloat16
    tokens = np.asarray(tokens)
    lstm_kernel = np.asarray(lstm_kernel, dtype=np.float32)
    lstm_bias = np.asarray(lstm_bias, dtype=np.float32)
    W1 = np.asarray(W1, dtype=np.float32)
    b1 = np.asarray(b1, dtype=np.float32)
    W2 = np.asarray(W2, dtype=np.float32)
    b2 = np.asarray(b2, dtype=np.float32)

    Wx = lstm_kernel[:V]
    Wh = lstm_kernel[V:]
    bias = lstm_bias.copy()
    bias[2 * H:3 * H] += 1.0  # forget-gate bias (i, j, f, o layout)

    # permuted z-dim order: dim' = (c*4+g)*128 + p  ->  GATE_BASE[g] + c*128 + p
    perm = np.empty(4 * H, dtype=np.int64)
    for c in range(4):
        for g in range(4):
            mt = c * 4 + g
            perm[mt * 128:(mt + 1) * 128] = GATE_BASE[g] + c * 128 + np.arange(128)

    # E with bias folded, gathered per core:
    # e[p, t*32 + c*8 + g*2 + b] = (Wx[tok[b,t]] + bias)[GATE_BASE[g] + c*128 + p]
    Wx_adj = (Wx + bias[None, :]).astype(bf)          # [V, 4H]
    Wx_re = np.ascontiguousarray(Wx_adj[:, perm])     # [V, (c,g,p) = ((c*4+g)*128+p)]

    # wh[p, (c*16 + ck*4 + g)*128 + pm] = Wh[ck*128+p, GATE_BASE[g] + c*128 + pm]
    wh = np.empty((128, 64 * 128), dtype=bf)
    Whb = Wh.astype(bf)
    for c in range(4):
        for ck in range(4):
            for g in range(4):
                slot = c * 16 + ck * 4 + g
                wh[:, slot * 128:(slot + 1) * 128] = Whb[
                    ck * 128:(ck + 1) * 128, GATE_BASE[g] + c * 128: GATE_BASE[g] + (c + 1) * 128
                ]

    # w1[p, c*DH + d] = (W1 * BN_S)[c*128 + p, d]
    W1s = (W1 * BN_S).astype(bf)
    w1 = np.empty((128, 4 * DH), dtype=bf)
    for c in range(4):
        w1[:, c * DH:(c + 1) * DH] = W1s[c * 128:(c + 1) * 128, :]

    # W2 with BN scale folded; b2 applied separately in fp32
    W2s = (W2 * BN_S).astype(bf)
    w2 = np.zeros((128, 2 * 4096), dtype=bf)
    w2[:, :4096] = W2s[0:128, :]
    w2[0:72, 4096:] = W2s[128:200, :]

    b1v = np.zeros((128, 2), dtype=np.float32)
    b1v[:, 0] = b1[0:128]
    b1v[0:72, 1] = b1[128:200]
    b2v = np.ascontiguousarray((b2 * BN_S).reshape(32, 128).T.astype(np.float32))

    in_maps = []
    for k in range(N_CORES):
        tok_core = tokens[2 * k:2 * k + 2, :T].astype(np.int64)  # [2, T]
        # [2, T, 2048] -> e[p, t, c, g, b]
        g_ = Wx_re[tok_core.reshape(-1)].reshape(2, T, 16, 128)   # [b, t, (c,g), p]
        e = np.ascontiguousarray(np.transpose(g_, (3, 1, 2, 0))).reshape(128, T * 32)
        in_maps.append({
            "e": e.astype(bf),
            "wh": wh,
            "w1": w1,
            "w2": w2,
            "b1v": b1v,
            "b2v": b2v,
        })
    return in_maps


def kernel(tokens, lstm_kernel, lstm_bias, W1, b1, W2, b2):
    from concourse.bass_utils import run_bass_kernel_spmd

    if "nc" not in _CACHE:
        _CACHE["nc"] = _build_program()
    nc = _CACHE["nc"]

    in_maps = _prep_host(tokens, lstm_kernel, lstm_bias, W1, b1, W2, b2)
    res = run_bass_kernel_spmd(nc, in_maps, list(range(N_CORES)))
    results = res.results

    out = np.empty((B * T, V), dtype=np.float32)
    for k in range(N_CORES):
        o = np.asarray(results[k]["out"], dtype=np.float32)  # [4096, 2T] (v, t*2+b)
        o = o.reshape(V, T, 2)
        out[(2 * k) * T:(2 * k + 1) * T, :] = o[:, :, 0].T
        out[(2 * k + 1) * T:(2 * k + 2) * T, :] = o[:, :, 1].T
    return out
